# revision 35
# baseline (speedup 1.0000x reference)
"""MultiHeadAttention Trainium2 kernel (v3).

Sharding: 8 cores = 4 batches x 2 head-groups (8 heads each).
Each core computes, for its (batch b, head group gi):
  Q = q[b] @ Wq[:, gi*512:+512] + bq_g        (same fc applied to k, v)
  per head: softmax(QK^T/8 with mask) @ V
  partial_out = attn @ Wo[gi*512:+512, :]
Host sums the two partial outputs per batch and adds b_o.

v3: the whole program is emitted as one software-pipelined stream (engine
execution follows emission order), paced by the ACT exp stream:
  - flattened beat loop over (chunk, head-pair, sk-tile); scores run 2
    beats ahead of exp, attnV runs LAG=4 beats behind (probs ring bufs=8)
    so pr/chunk boundaries never drain the pipeline
  - K projection (d-tiles 1-3), Q projection of the next chunk and the
    output projection of the previous chunk are emitted as "pieces" into
    fixed drain slots (t==9) inside the beat stream, filling PE slack
  - x tiles for K are RE-LOADED during chunk 0 (xk2) so the piece
    projections don't force 24 x-tiles resident at once
  - mask tiles prefetched one full head-pair ahead (Pool/SWDGE queue,
    2-tile groups emitted spread across the previous pr)
  - normalization via DVE InstReciprocal (HW-verified exact); single act
    table load (Identity+Exp share a table)
  - x/Wq in bf16 (halves x DMA; measured +3e-3 rel err, gate is 2e-2)
  - output projection in [128,1024] psum pieces, PSUM->SBUF copy on the
    Pool engine, store DMA on SP

Device layout (per core):
  - inputs arrive TRANSPOSED: xT [1024, seq] bf16
  - Q^T, K^T stored [128, 4, *] bf16 (partition = d within d-tile; head g
    occupies partitions 64*(g%2).. of d-tile g//2)
  - scores computed transposed S^T[sk, sq], two heads packed in the PE
    array via tile_position row tiling (K=64 each)
  - softmax without max-subtraction (scores bounded ~|6| after 1/8 scale)
  - mask applied multiplicatively AFTER exp (notmask in {0,1} bf16)
  - attn@V uses lhsT=[V_head | ones] (M=65): row 64 accumulates the
    softmax denominator for free
"""

import sys

import numpy as np
import ml_dtypes

try:
    import concourse.bass as bass  # noqa: F401
except ImportError:  # pragma: no cover
    for _p in ("/opt/trn_rl_repo", "/root/.axon_site/_ro/trn_rl_repo"):
        if _p not in sys.path:
            sys.path.insert(0, _p)
    import concourse.bass as bass  # noqa: F401

import concourse.tile as tile
from concourse import bacc, mybir
from concourse.bass_utils import run_bass_kernel_spmd

BF16 = ml_dtypes.bfloat16

D_MODEL = 1024
N_HEADS = 16
BATCH = 4
SEQ = 2048
DH = 64           # head dim
HG = 8            # heads per core
DG = HG * DH      # 512, projected dim per core

F32 = mybir.dt.float32
BF16D = mybir.dt.bfloat16

TG = 2            # sk tiles per mask DMA group
LAG = 4           # attnV beats behind exp


def build_nc(seq=SEQ):
    """Build the per-core SPMD Bass program."""
    assert seq % 512 == 0
    NT = seq // 128       # sk tiles (16)
    NC_ = seq // 512      # sq chunks (4)
    NG = NT // TG         # mask groups per (c, pr) (8)
    NB = NC_ * 4 * NT     # total beats (256)
    BEATS = [(c, pr, t) for c in range(NC_) for pr in range(4)
             for t in range(NT)]

    nc = bacc.Bacc(None, target_bir_lowering=False)

    xqT = nc.dram_tensor("xqT", [D_MODEL, seq], BF16D, kind="ExternalInput")
    xkT = nc.dram_tensor("xkT", [D_MODEL, seq], BF16D, kind="ExternalInput")
    xvT = nc.dram_tensor("xvT", [D_MODEL, seq], BF16D, kind="ExternalInput")
    wq = nc.dram_tensor("wq", [D_MODEL, DG], BF16D, kind="ExternalInput")
    bq = nc.dram_tensor("bq", [DG], F32, kind="ExternalInput")
    wo = nc.dram_tensor("wo", [DG, D_MODEL], BF16D, kind="ExternalInput")
    # notmask, transposed + tiled: [c, pr, g, p(sk in tile), t(in group), h, j]
    nm = nc.dram_tensor("nm", [NC_, 4, NG, 128, TG, 2, 512], BF16D,
                        kind="ExternalInput")
    out = nc.dram_tensor("out", [seq, D_MODEL], F32, kind="ExternalOutput")

    IDENT = mybir.ActivationFunctionType.Identity
    EXP = mybir.ActivationFunctionType.Exp

    with tile.TileContext(nc) as tc:
        with tc.tile_pool(name="persist", bufs=1) as persist, \
             tc.tile_pool(name="qtp", bufs=2) as qtp, \
             tc.tile_pool(name="atp", bufs=2) as atp, \
             tc.tile_pool(name="xpool", bufs=16) as xpool, \
             tc.tile_pool(name="nmp", bufs=9) as nmp, \
             tc.tile_pool(name="probsp", bufs=8) as probsp, \
             tc.tile_pool(name="tmpp", bufs=2) as tmpp, \
             tc.tile_pool(name="rp", bufs=1) as rp, \
             tc.tile_pool(name="osbp", bufs=2) as osbp, \
             tc.tile_pool(name="mmps", bufs=2, space="PSUM") as mmps, \
             tc.tile_pool(name="accp", bufs=2, space="PSUM") as accp:

            kt_sb = persist.tile([128, 4, seq], BF16D, name="kt_sb")
            v_sb = persist.tile([128, NT, HG, DH + 1], BF16D, name="v_sb")
            wo_sb = persist.tile([128, 4, D_MODEL], BF16D, name="wo_sb")
            wq_sb = persist.tile([128, 8, DG], BF16D, name="wq_sb")
            bq_sb = persist.tile([128, 4], F32, name="bq_sb")
            bqrep = persist.tile([128, HG, DH], F32, name="bqrep")

            nc.sync.dma_start(out=wq_sb, in_=wq.rearrange("(n p) m -> p n m", p=128))
            nc.sync.dma_start(out=bq_sb, in_=bq.rearrange("(t p) -> p t", p=128))
            _bqap = bq[:].rearrange("(g e) -> g e", g=HG)
            nc.gpsimd.dma_start(out=bqrep, in_=bass.AP(
                tensor=_bqap.tensor, offset=_bqap.offset,
                ap=[[0, 128]] + [list(d) for d in _bqap.ap]))
            nc.sync.dma_start(out=wo_sb, in_=wo.rearrange("(n p) m -> p n m", p=128))
            nc.vector.memset(v_sb[:, :, :, DH:DH + 1], 1.0)

            def load_x(xh, pref):
                ts_ = []
                for db in range(8):
                    xt = xpool.tile([128, seq], BF16D, name=f"{pref}{db}", tag="x")
                    nc.gpsimd.dma_start(out=xt, in_=xh[db * 128:(db + 1) * 128, :])
                    ts_.append(xt)
                return ts_

            # projection piece emitters ------------------------------------
            def _ppool(tag):
                return mmps if tag == "mm" else accp

            def kpiece(dt, half, xts, width=512, tag="mm"):
                """K^T projection for d-tile dt, `width` seq cols from
                half*width (one [128,width] psum)."""
                h0 = half * width
                ps = _ppool(tag).tile([128, width], F32, name="kps", tag=tag)
                for db in range(8):
                    nc.tensor.matmul(
                        ps,
                        wq_sb[:, db, dt * 128:(dt + 1) * 128],
                        xts[db][:, h0:h0 + width],
                        start=(db == 0), stop=(db == 7),
                    )
                nc.scalar.activation(
                    kt_sb[:, dt, h0:h0 + width], ps,
                    IDENT, bias=bq_sb[:, dt:dt + 1], scale=1.0,
                )

            qt_d = {}

            def _qt(c):
                if c not in qt_d:
                    qt_d[c] = qtp.tile([128, 4, 512], BF16D,
                                       name=f"qt{c}", tag="qt")
                return qt_d[c]

            def qpiece(c, dt, xts, half=None, tag="mm"):
                """Q projection for chunk c, one d-tile (or a 256-wide half)."""
                j0 = 0 if half is None else half * 256
                w = 512 if half is None else 256
                cs = slice(c * 512 + j0, c * 512 + j0 + w)
                qt_c = _qt(c)
                ps = _ppool(tag).tile([128, w], F32, name="qps", tag=tag)
                for db in range(8):
                    nc.tensor.matmul(
                        ps,
                        wq_sb[:, db, dt * 128:(dt + 1) * 128],
                        xts[db][:, cs],
                        start=(db == 0), stop=(db == 7),
                    )
                nc.vector.tensor_scalar_add(
                    qt_c[:, dt, j0:j0 + w], ps, bq_sb[:, dt:dt + 1],
                )

            def qpair_parts(c, dts, xts, tag="acc"):
                """Q projection for chunk c, two d-tiles sharing one
                [128,1024] psum; 4 part-closures (~853ns PE each: half the
                db contraction per part) for spreading over window beats."""
                cs = slice(c * 512, (c + 1) * 512)
                state = {}

                def part(i, dt, dbh):
                    if "ps" not in state:
                        state["qt"] = _qt(c)
                        state["ps"] = _ppool(tag).tile(
                            [128, 1024], F32, name="qps2", tag=tag)
                    ps = state["ps"]
                    for db in range(4 * dbh, 4 * dbh + 4):
                        nc.tensor.matmul(
                            ps[:, i * 512:(i + 1) * 512],
                            wq_sb[:, db, dt * 128:(dt + 1) * 128],
                            xts[db][:, cs],
                            start=(db == 0), stop=(db == 7),
                        )
                    if dbh == 1:
                        nc.vector.tensor_scalar_add(
                            state["qt"][:, dt, :], ps[:, i * 512:(i + 1) * 512],
                            bq_sb[:, dt:dt + 1],
                        )
                return [lambda i=i, dt=dt, dbh=dbh: part(i, dt, dbh)
                        for i, dt in enumerate(dts) for dbh in range(2)]

            def vpiece(st, xts):
                """V projection for one sk tile (one [128,512] psum)."""
                ps = mmps.tile([128, DG], F32, name="vps", tag="mm")
                for db in range(8):
                    nc.tensor.matmul(
                        ps, xts[db][:, st * 128:(st + 1) * 128], wq_sb[:, db, :],
                        start=(db == 0), stop=(db == 7),
                    )
                nc.vector.tensor_add(
                    v_sb[:, st, :, 0:DH],
                    ps[:, :].rearrange("p (g e) -> p g e", g=HG),
                    bqrep,
                )

            at_d = {}

            def opiece(c, k, dch, tag="mm"):
                """Output projection rows [c*512+k*128, +128) x 512 cols."""
                row = c * 512 + k * 128
                ops = _ppool(tag).tile([128, 512], F32, name="ops", tag=tag)
                for pr in range(4):
                    nc.tensor.matmul(
                        ops,
                        at_d[c][:, pr, k * 128:(k + 1) * 128],
                        wo_sb[:, pr, dch * 512:(dch + 1) * 512],
                        start=(pr == 0), stop=(pr == 3),
                    )
                osb = osbp.tile([128, 512], F32, name="osb", tag="osb")
                nc.gpsimd.tensor_copy(osb, ops)
                nc.sync.dma_start(
                    out=out[row:row + 128, dch * 512:(dch + 1) * 512], in_=osb)

            def ofull_parts(c, k, tag="acc"):
                """Output projection rows [c*512+k*128, +128), all 1024 cols
                sharing one [128,1024] psum; 2 part-closures (~0.85us each)."""
                row = c * 512 + k * 128
                state = {}

                def part(dch):
                    if "ps" not in state:
                        state["ps"] = _ppool(tag).tile(
                            [128, 1024], F32, name="opsf", tag=tag)
                    ops = state["ps"]
                    for pr in range(4):
                        nc.tensor.matmul(
                            ops[:, dch * 512:(dch + 1) * 512],
                            at_d[c][:, pr, k * 128:(k + 1) * 128],
                            wo_sb[:, pr, dch * 512:(dch + 1) * 512],
                            start=(pr == 0), stop=(pr == 3),
                        )
                    if dch == 1:
                        osb = osbp.tile([128, 1024], F32, name="osbf",
                                        tag="osb")
                        nc.gpsimd.tensor_copy(osb, ops)
                        nc.sync.dma_start(out=out[row:row + 128, :], in_=osb)
                return [lambda dch=dch: part(dch) for dch in range(2)]

            # ---- lead-in --------------------------------------------------
            xk_ts = load_x(xkT, "xk")
            xv_ts = load_x(xvT, "xv")
            xq_ts = load_x(xqT, "xq")

            nmt_d = {p: [None] * NG for p in range(16)}

            def emit_nmt_group(p, g):
                c, pr = divmod(p, 4)
                nmt = nmp.tile([128, TG, 2, 512], BF16D, name="nmt", tag="nmt")
                nc.gpsimd.dma_start(out=nmt, in_=nm[c, pr, g])
                nmt_d[p][g] = nmt

            for quarter in range(4):
                kpiece(0, quarter, xk_ts)
            for st in range(NT):
                vpiece(st, xv_ts)
            for dt in range(4):
                qpiece(0, dt, xq_ts)
            for g in range(NG):
                emit_nmt_group(0, g)
            xk2_ts = load_x(xkT, "xk2")   # re-load for K d-tiles 1-3 pieces

            # ---- flattened beat stream -----------------------------------
            spair_d, probs_d, acc_d = {}, {}, {}

            def emit_s(b):
                c, pr, t = BEATS[b]
                cs = slice(c * 512, (c + 1) * 512)
                sp = mmps.tile([128, 2, 512], F32, name="spair", tag="mm")
                spair_d[b] = sp
                tc_cols = slice(t * 128, (t + 1) * 128)
                nc.tensor.matmul(
                    sp[:, 0, :], kt_sb[0:64, pr, tc_cols],
                    qt_d[c][0:64, pr, :], start=True, stop=True,
                    tile_position=(0, 0),
                )
                nc.tensor.matmul(
                    sp[:, 1, :], kt_sb[64:128, pr, tc_cols],
                    qt_d[c][64:128, pr, :], start=True, stop=True,
                    tile_position=(64, 0),
                )

            def emit_exp_mask(b):
                c, pr, t = BEATS[b]
                p = 4 * c + pr
                probs = probsp.tile([128, 2, 512], BF16D, name="probs",
                                    tag="probs")
                probs_d[b] = probs
                nc.scalar.activation(probs, spair_d[b], EXP, scale=0.125)
                nc.vector.tensor_mul(probs, probs,
                                     nmt_d[p][t // TG][:, t % TG, :, :])

            def emit_av(b):
                c, pr, t = BEATS[b]
                p = 4 * c + pr
                if t == 0:
                    acc_d[p] = accp.tile([DH + 1, 2, 512], F32, name="acc",
                                         tag="acc")
                acc = acc_d[p]
                for h in range(2):
                    nc.tensor.matmul(
                        acc[:, h, :], v_sb[:, t, 2 * pr + h, :],
                        probs_d[b][:, h, :], start=(t == 0), stop=(t == NT - 1),
                    )

            def emit_norm(c, pr):
                p = 4 * c + pr
                acc = acc_d[p]
                if c not in at_d:
                    at_d[c] = atp.tile([128, 4, 512], BF16D,
                                       name=f"at{c}", tag="at")
                at_c = at_d[c]
                rsb = rp.tile([1, 2, 512], F32, name="rsb", tag="rsb")
                nc.vector.reciprocal(rsb, acc[DH:DH + 1, :, :])
                rrep = rp.tile([64, 2, 512], F32, name="rrep", tag="rrep")
                nc.gpsimd.partition_broadcast(rrep, rsb)
                tmpt = tmpp.tile([64, 2, 512], BF16D, name="tmpt", tag="tmpt")
                nc.vector.tensor_mul(tmpt, acc[0:DH, :, :], rrep)
                nc.sync.dma_start(out=at_c[0:64, pr, :], in_=tmpt[:, 0, :])
                nc.sync.dma_start(out=at_c[64:128, pr, :], in_=tmpt[:, 1, :])

            big, small = [], []

            def drain1():
                if small:
                    small.pop(0)(tag="mm")

            next_s = 0
            for b in range(NB + LAG):
                if b < NB:
                    c, pr, t = BEATS[b]
                    p = 4 * c + pr
                    if t == 0 and pr == 0:
                        # queue this chunk's deferred pieces: "big" ones run
                        # in the per-pr acc-ring window (t==8, zero spair-ring
                        # conflict), small leftovers go to mm drain slots
                        if c == 0:
                            # chunk 0 is PE-bound (ACT runs ahead), so mm
                            # drain slots are cheap there: K d-tiles 1-3 go
                            # through them; Q(c1) takes two acc windows
                            for dt in (1, 2, 3):
                                for e in range(8):
                                    small.append(
                                        lambda dt=dt, e=e, tag="mm":
                                        kpiece(dt, e, xk2_ts, width=256, tag=tag))
                            big.append(qpair_parts(1, (0, 1), xq_ts))
                            big.append(qpair_parts(1, (2, 3), xq_ts))
                        else:
                            if c + 1 < NC_:
                                big.append(qpair_parts(c + 1, (0, 1), xq_ts))
                                big.append(qpair_parts(c + 1, (2, 3), xq_ts))
                            for kk in (0, 1):
                                big.append(ofull_parts(c - 1, kk))
                            for kk in (2, 3):
                                for dch in range(2):
                                    small.append(
                                        lambda c=c, kk=kk, dch=dch, tag="mm":
                                        opiece(c - 1, kk, dch, tag=tag))
                    # mask prefetch, one pr ahead, spread over even beats
                    if p + 1 < 16:
                        if t == 0:
                            emit_nmt_group(p + 1, 0)
                            emit_nmt_group(p + 1, 1)
                        elif t % 2 == 0 and t <= 12:
                            emit_nmt_group(p + 1, t // 2 + 1)
                    # small pieces drain into mm slots (cheap while chunk 0
                    # is PE-bound; rationed later). In pr0 of chunks past the
                    # first, delay slots so the previous chunk's norms
                    # (emitted LAG beats in) land first.
                    if c == 0:
                        slot_ok = t % 2 == 1
                    elif pr == 0:
                        slot_ok = t in (7, 11)
                    else:
                        slot_ok = t in (3, 11)
                    is_slot = slot_ok and small
                    if not is_slot:
                        # scores run 2 beats ahead of the exp stream
                        while next_s <= b + 2 and next_s < NB:
                            emit_s(next_s)
                            next_s += 1
                    emit_exp_mask(b)
                    if is_slot:
                        # one piece psum reuses a spair ring slot whose exp
                        # is in flight right now; scores catch up next beat
                        drain1()
                    if t in (6, 9, 12, 15) and big:
                        # the pr's acc-ring slot is free between the previous
                        # pr's norm and the next pr's accumulator: run one
                        # big piece there (split over two beats so the exp
                        # cushion absorbs each part) with no spair conflict
                        parts = big[0]
                        parts.pop(0)()
                        if not parts:
                            big.pop(0)
                if b >= LAG:
                    bb = b - LAG
                    emit_av(bb)
                    cb, prb, tb = BEATS[bb]
                    if tb == NT - 1:
                        emit_norm(cb, prb)
            while big:
                for part in big.pop(0):
                    part()
            while small:
                small.pop(0)(tag="mm")
            # tail: outproj of last chunk, alternating PSUM pools for depth
            for kk in range(4):
                for part in ofull_parts(NC_ - 1, kk,
                                        tag="mm" if kk % 2 == 0 else "acc"):
                    part()

    nc.compile()
    return nc


_NC_CACHE = {}


def _get_nc(seq=SEQ):
    if seq not in _NC_CACHE:
        _NC_CACHE[seq] = build_nc(seq)
    return _NC_CACHE[seq]


def make_core_inputs(q, k, v, mask, W_q, b_q, W_o, seq=SEQ):
    """Build the 8 per-core input maps (host-side shard + layout)."""
    NT = seq // 128
    NC_ = seq // 512
    NG = NT // TG
    in_maps = []
    notm_all = (~np.asarray(mask)).astype(BF16)  # [B, 16, sq, sk]
    for core in range(8):
        b, gi = divmod(core, 2)
        cols = slice(gi * DG, (gi + 1) * DG)
        xqT = np.ascontiguousarray(np.asarray(q[b], np.float32).T.astype(BF16))
        xkT = np.ascontiguousarray(np.asarray(k[b], np.float32).T.astype(BF16))
        xvT = np.ascontiguousarray(np.asarray(v[b], np.float32).T.astype(BF16))
        wqc = np.ascontiguousarray(np.asarray(W_q, np.float32)[:, cols]).astype(BF16)
        bqc = np.ascontiguousarray(np.asarray(b_q, np.float32)[cols])
        woc = np.ascontiguousarray(np.asarray(W_o, np.float32)[cols, :]).astype(BF16)
        nmc = notm_all[b, gi * HG:(gi + 1) * HG]  # [8, sq, sk] bf16
        # [8h, sq, sk] -> [c, pr, g, p, t, h, j]
        # h -> (pr 4, h2); sq -> (c 4, j 512); sk -> (g NG, t TG, p 128)
        nmc = np.ascontiguousarray(
            nmc.reshape(4, 2, NC_, 512, NG, TG, 128)
               .transpose(2, 0, 4, 6, 5, 1, 3)
        )
        in_maps.append({
            "xqT": xqT, "xkT": xkT, "xvT": xvT,
            "wq": wqc, "bq": bqc, "wo": woc, "nm": nmc,
        })
    return in_maps


def kernel(q, k, v, mask, W_q, b_q, W_o, b_o):
    nc = _get_nc(SEQ)
    in_maps = make_core_inputs(q, k, v, mask, W_q, b_q, W_o, SEQ)
    res = run_bass_kernel_spmd(nc, in_maps, core_ids=list(range(8)))
    out = np.empty((BATCH, SEQ, D_MODEL), np.float32)
    bo = np.asarray(b_o, np.float32)
    for b in range(BATCH):
        out[b] = res.results[2 * b]["out"] + res.results[2 * b + 1]["out"] + bo
    return out
